# revision 26
# baseline (speedup 1.0000x reference)
"""Trainium2 Bass kernel for nn_BaseCompressor2 (truncated one-pole IIR compressor).

Algorithm (per batch n, signal length L=262144, C=2 channels):
  energy[t] = mean_c(sig[c,t]^2)
  y = IIR(energy): y[t] = alpha*y[t-1] + (1-alpha)*energy[t]
  x = ln(y + 1e-5); piecewise knee gain; out = exp(log_gain) * sig

Mapping: batch N=32 sharded 4-per-core across 8 cores (pure data parallel).
Per batch: [128 partitions x 2048] tiles, partition p = time block
[p*2048,(p+1)*2048).

Engine plan:
  - bf16 signal I/O (host casts; total error ~0.6% << 2e-2 gate): halves DMA
    traffic and doubles DVE rate on the output muls.
  - Act: 4 passes/batch, all from ONE table set (Square(s01) 4096-wide, Ln,
    Square(knee), Exp). Per-batch constants ride the Act free affine:
      x_hat = Ln((1-a)/2*e^-thr * y + 1e-5*e^-thr)  = log_energy - thr
      q3    = Square(vc/sqrt(2k) + sqrt(k/8))       = k*v^2/2, v=(x_hat+k/2)/k
      gain  = Exp(-|c1| * t)                        (ratio folded into scale)
    get_activation_tables is wrapped so the table-load pass resolves all of
    Square/Ln/Exp to natural_log_exp_and_others (canonical id preserved):
    exactly one ACT_TABLE_LOAD.
  - DVE: e=sq0+sq1 (bf16 TT, 2x), the scan (fp32 state; bf16 data), the tiny
    carry-stt, vc/d clamps as fp16 2-op tensor_scalar (4x_2P), t=q3+d (fp16
    TT 2x), both output muls (bf16 TT 2x). fp16 (not bf16) on the post-Ln
    chain: 11-bit mantissa keeps the knee error ~0.2%.
  - PE: only the 1-column carry shift-matmul. PSUM holds just C [128,1].
  - GpSimd: no compute (its Q7 SBUF traffic slows concurrent DVE scans by
    ~50%) - it only issues output DMAs on the SWDGE ring. Input DMAs are all
    front-loaded on the sync HWDGE ring.
  - Software-pipelined emission A0 A1 B0 A2 B1 A3 B2 B3 (A = square/e/scan/
    carry, B = ln/knee/gain/muls/store): engine queues are FIFO in emission
    order, so batch-sequential emission would head-of-line-block early work
    of batch b+1 behind late work of batch b.

Knee identity (exact vs reference up to the 0.001 regularizers):
  vc = clamp(x_hat, -k/2, k/2);  d = relu(x_hat - k/2)
  t  = k*((vc + k/2)/k)^2/2 + d;  log_gain = -|c1| * t
  middle: -|c1|*(x-thr+k/2)^2/(2k); above: -|c1|*(x-thr) exact; below: 0.
"""

import numpy as np

N, C, L = 32, 2, 262144
NCORES = 8
BPC = N // NCORES  # batches per core
P = 128
FD = L // P   # 2048 free elems per partition
H = FD // 2   # scan half

# pcols column layout (per batch b, base b*NP)
NP = 8
ALPHA, LNSC, LNB, PKH, NKH, QSA, QSB, NEGC1 = range(NP)

_cache = {}


def _host_params(z_alpha_pre, log_threshold, log_ratio, log_knee):
    """Per-batch derived scalars, float64 math -> float32 columns."""
    z = z_alpha_pre.astype(np.float64).reshape(-1)
    thr = log_threshold.astype(np.float64).reshape(-1) - 6.0
    knee = np.exp(log_knee.astype(np.float64).reshape(-1))
    r001 = 1.0 + np.exp(log_ratio.astype(np.float64).reshape(-1)) + 0.001
    alpha = 1.0 / (1.0 + np.exp(-z))
    # Carry-truncation validity: alpha^2048 must underflow to exactly 0 in f32.
    assert np.max(2048.0 * np.log(alpha)) < -88.0, "alpha too close to 1"
    ac1 = 1.0 - 1.0 / r001  # |c1| > 0
    emt = np.exp(-thr)
    vals = np.zeros((N, NP), dtype=np.float64)
    vals[:, ALPHA] = alpha
    vals[:, LNSC] = (1.0 - alpha) / 2.0 * emt
    vals[:, LNB] = 1e-5 * emt
    vals[:, PKH] = knee / 2.0
    vals[:, NKH] = -knee / 2.0
    vals[:, QSA] = 1.0 / np.sqrt(2.0 * knee)
    vals[:, QSB] = np.sqrt(knee / 8.0)
    vals[:, NEGC1] = -ac1
    # carry influence horizon: alpha^(t+1) < 1e-9 for t >= T0
    t0 = int(np.ceil(20.8 / max(1e-9, -np.max(np.log(alpha))))) + 32
    t0 = min(FD, max(64, t0))
    return vals.astype(np.float32), t0


def _shift_matrix():
    # lhsT[k, m] = 1 iff m == k+1, so (lhsT.T @ f)[m] = f[m-1], row 0 -> 0
    m = np.zeros((P, P), dtype=np.float32)
    m[np.arange(P - 1), np.arange(1, P)] = 1.0
    return m


def _pin_act_tables():
    """Make the act-table-load pass resolve every activation to the
    natural_log_exp_and_others set (it contains square+ln+exp): empty out
    all other sets while preserving list order, so the emitted
    act_func_set_id keeps its canonical index."""
    import concourse.bacc as bacc
    import concourse.hw_specs as hw_specs

    orig = hw_specs.get_activation_tables

    def pinned(arch, _orig=orig):
        items = list(_orig(arch).items())
        keep = "natural_log_exp_and_others"
        return {n: (fns if n == keep else set()) for n, fns in items}

    bacc.get_activation_tables = pinned


def _build_program(T0):
    from contextlib import ExitStack

    import concourse.bacc as bacc
    import concourse.bass as bass
    import concourse.tile as tile
    from concourse import mybir

    _pin_act_tables()

    f32 = mybir.dt.float32
    f16 = mybir.dt.float16
    bf16 = mybir.dt.bfloat16
    Alu = mybir.AluOpType
    Af = mybir.ActivationFunctionType

    nc = bacc.Bacc(
        "TRN2", target_bir_lowering=False, debug=False,
        enable_asserts=False, num_devices=NCORES,
    )
    sig = nc.dram_tensor("sig", [BPC, C, L], bf16, kind="ExternalInput")
    pcols = nc.dram_tensor("pcols", [P, BPC * NP], f32, kind="ExternalInput")
    shiftm = nc.dram_tensor("shiftm", [P, P], f32, kind="ExternalInput")
    idmat = nc.dram_tensor("idmat", [P, P], bf16, kind="ExternalInput")
    pwt = nc.dram_tensor("pwt", [BPC, T0], f32, kind="ExternalInput")
    out = nc.dram_tensor("out", [BPC, C, L], bf16, kind="ExternalOutput")

    with tile.TileContext(nc) as tc, ExitStack() as ctx:
        const = ctx.enter_context(tc.tile_pool(name="const", bufs=1))
        io = ctx.enter_context(tc.tile_pool(name="io", bufs=4))
        wka = ctx.enter_context(tc.tile_pool(name="wka", bufs=2))
        ypool = ctx.enter_context(tc.tile_pool(name="ypool", bufs=3))
        wkb = ctx.enter_context(tc.tile_pool(name="wkb", bufs=2))
        opool = ctx.enter_context(tc.tile_pool(name="opool", bufs=2))
        psum = ctx.enter_context(tc.tile_pool(name="psum", bufs=2, space="PSUM"))

        # consts first (tiny, and scan(0)/carry(0) depend on them), then all
        # signal inputs on the fast sync HWDGE ring (the SWDGE ring measures
        # ~69 GB/s vs ~265 GB/s for HWDGE). Batch 0 goes per channel so
        # squaring can start when ch0 lands; batches 1..3 as one 3D transfer
        # each.
        pc = const.tile([P, BPC * NP], f32, tag="pc")
        nc.sync.dma_start(pc, pcols.ap())
        shm = const.tile([P, P], f32, tag="shm")
        nc.sync.dma_start(shm, shiftm.ap())
        idm = const.tile([P, P], bf16, tag="idm")
        nc.sync.dma_start(idm, idmat.ap())
        # alpha-power table for all batches, partition-broadcast, SWDGE queue
        pw_all = const.tile([P, BPC, T0], f32, tag="pw")
        nc.gpsimd.dma_start(pw_all, bass.AP(pwt, 0, [[0, P], [T0, BPC], [1, T0]]))

        s01s, ys = {}, {}
        for b in range(BPC):
            s01 = io.tile([P, C, FD], bf16, tag="s01")
            s01s[b] = s01
        # batch 0 in channel-half quarters so compute ramps in ASAP
        for h in range(2):
            sl = slice(h * H, (h + 1) * H)
            for c in range(C):
                nc.sync.dma_start(
                    s01s[0][:, c, sl],
                    sig.ap()[0, c].rearrange("(p f) -> p f", p=P)[:, sl])
        for b in range(1, BPC):
            nc.sync.dma_start(
                s01s[b], sig.ap()[b].rearrange("c (p f) -> p c f", p=P))

        def col(b, j):
            return pc[:, b * NP + j: b * NP + j + 1]

        def phase_a(b):
            s01 = s01s[b]
            sq01 = wka.tile([P, C, FD], bf16, tag="sq01")
            e = wka.tile([P, FD], bf16, tag="e")
            y = ypool.tile([P, FD], f32, tag="y")
            if b == 0:
                # quarter/half-granular ramp-in matching the quarter DMAs
                for h in range(2):
                    sl = slice(h * H, (h + 1) * H)
                    for c in range(C):
                        nc.scalar.activation(sq01[:, c, sl], s01[:, c, sl],
                                             Af.Square)
                    nc.vector.tensor_add(e[:, sl], sq01[:, 0, sl],
                                         sq01[:, 1, sl])
                    init = 0.0 if h == 0 else y[:, H - 1: H]
                    nc.vector.tensor_tensor_scan(
                        y[:, sl], col(b, ALPHA).to_broadcast((P, H)),
                        e[:, sl], init, Alu.mult, Alu.add)
            else:
                nc.scalar.activation(
                    sq01.rearrange("p c f -> p (c f)"),
                    s01.rearrange("p c f -> p (c f)"), Af.Square)
                nc.vector.tensor_add(e, sq01[:, 0], sq01[:, 1])
                nc.vector.tensor_tensor_scan(
                    y, col(b, ALPHA).to_broadcast((P, FD)),
                    e, 0.0, Alu.mult, Alu.add)
            # carry C[p] = y[p-1, FD-1] via shift-matmul into a spare column
            # of the t PSUM tile (consumed by the stt below, overwritten by
            # the t-add later); y += pw * C
            t_ps = psum.tile([P, FD], f32, tag="t")
            nc.tensor.matmul(t_ps[:, FD - 1: FD], shm, y[:, FD - 1: FD],
                             start=True, stop=True)
            nc.vector.scalar_tensor_tensor(
                y[:, 0:T0], pw_all[:, b, :], t_ps[:, FD - 1: FD],
                y[:, 0:T0], Alu.mult, Alu.add)
            ys[b] = (y, t_ps)

        def phase_b(b):
            s01, (y, t_ps) = s01s[b], ys[b]
            xh = wkb.tile([P, FD], f16, tag="xh")
            vc = wkb.tile([P, FD], f16, tag="vc")
            d = wkb.tile([P, FD], bf16, tag="d")
            q3 = wkb.tile([P, FD], bf16, tag="q3")
            g = wkb.tile([P, FD], bf16, tag="g")
            o01 = opool.tile([P, C, FD], bf16, tag="o01")
            ob = out.ap()[b].rearrange("c (p f) -> p c f", p=P)
            if b < BPC - 1:
                # x_hat = ln(energy + 1e-5) - thr (fp16)
                nc.scalar.activation(xh, y, Af.Ln, bias=col(b, LNB),
                                     scale=col(b, LNSC))
                # vc = clamp(x_hat,-k/2,k/2); d = relu(x_hat-k/2);
                # q3 = (vc/sqrt(2k)+sqrt(k/8))^2; t = q3+d on PE via
                # identity-matmul PSUM accumulation (bf16 inputs, f32 psum)
                nc.vector.tensor_scalar(vc, xh, col(b, NKH), col(b, PKH),
                                        Alu.max, Alu.min)
                nc.vector.tensor_scalar(d, xh, col(b, PKH), 0.0,
                                        Alu.subtract, Alu.max)
                nc.scalar.activation(q3, vc, Af.Square, bias=col(b, QSB),
                                     scale=col(b, QSA))
                for j in range(0, FD, 512):
                    nc.tensor.matmul(t_ps[:, j:j + 512], idm,
                                     q3[:, j:j + 512], start=True, stop=False)
                    nc.tensor.matmul(t_ps[:, j:j + 512], idm,
                                     d[:, j:j + 512], start=False, stop=True)
                nc.scalar.activation(g, t_ps, Af.Exp, scale=col(b, NEGC1))
                nc.vector.tensor_mul(o01[:, 0], g, s01[:, 0])
                nc.vector.tensor_mul(o01[:, 1], g, s01[:, 1])
                for c in range(C):
                    nc.sync.dma_start(ob[:, c], o01[:, c])
            else:
                # last batch: the whole chain is the pipeline tail, so run it
                # half-granular with the t-add on the (now idle) DVE and
                # drain outputs on BOTH HWDGE rings.
                t = wkb.tile([P, FD], f16, tag="t")
                for h in range(2):
                    sl = slice(h * H, (h + 1) * H)
                    nc.scalar.activation(xh[:, sl], y[:, sl], Af.Ln,
                                         bias=col(b, LNB), scale=col(b, LNSC))
                    nc.vector.tensor_scalar(vc[:, sl], xh[:, sl], col(b, NKH),
                                            col(b, PKH), Alu.max, Alu.min)
                    nc.vector.tensor_scalar(d[:, sl], xh[:, sl], col(b, PKH),
                                            0.0, Alu.subtract, Alu.max)
                    nc.scalar.activation(q3[:, sl], vc[:, sl], Af.Square,
                                         bias=col(b, QSB), scale=col(b, QSA))
                    nc.vector.tensor_add(t[:, sl], q3[:, sl], d[:, sl])
                    nc.scalar.activation(g[:, sl], t[:, sl], Af.Exp,
                                         scale=col(b, NEGC1))
                    nc.vector.tensor_mul(o01[:, 0, sl], g[:, sl],
                                         s01[:, 0, sl])
                    nc.sync.dma_start(ob[:, 0, sl], o01[:, 0, sl])
                    nc.vector.tensor_mul(o01[:, 1, sl], g[:, sl],
                                         s01[:, 1, sl])
                    nc.scalar.dma_start(ob[:, 1, sl], o01[:, 1, sl])

        # software pipeline: A0 A1 B0 A2 B1 A3 B2 B3
        phase_a(0)
        phase_a(1)
        for b in range(BPC):
            phase_b(b)
            if b + 2 < BPC:
                phase_a(b + 2)

    nc.compile()
    return nc


def _get_program(T0):
    key = ("nc", T0)
    if key not in _cache:
        _cache[key] = _build_program(T0)
    return _cache[key]


def _run(inputs, trace=False):
    import ml_dtypes
    from concourse.bass_utils import run_bass_kernel_spmd

    bf = ml_dtypes.bfloat16
    sig_full = np.asarray(inputs["input_signals"], np.float32).astype(bf)
    pv, T0 = _host_params(
        np.asarray(inputs["z_alpha_pre"], np.float32),
        np.asarray(inputs["log_threshold"], np.float32),
        np.asarray(inputs["log_ratio"], np.float32),
        np.asarray(inputs["log_knee"], np.float32),
    )

    nc = _get_program(T0)
    shm = _shift_matrix()
    idm_np = np.eye(P, dtype=np.float32).astype(bf)
    zf = np.asarray(inputs["z_alpha_pre"], np.float64).reshape(-1)
    alpha64 = 1.0 / (1.0 + np.exp(-zf))
    tpow = np.arange(1, T0 + 1, dtype=np.float64)
    pw_np = np.exp(tpow[None, :] * np.log(alpha64)[:, None]).astype(np.float32)
    in_maps = []
    for k in range(NCORES):
        shard = np.ascontiguousarray(sig_full[k * BPC:(k + 1) * BPC])
        cols = np.broadcast_to(
            pv[k * BPC:(k + 1) * BPC].reshape(1, BPC * NP), (P, BPC * NP)
        )
        in_maps.append({"sig": shard, "pcols": np.ascontiguousarray(cols),
                        "shiftm": shm, "idmat": idm_np,
                        "pwt": np.ascontiguousarray(pw_np[k * BPC:(k + 1) * BPC])})

    res = run_bass_kernel_spmd(
        nc, in_maps, core_ids=list(range(NCORES)), trace=trace,
    )
    out = np.empty((N, C, L), dtype=np.float32)
    for k in range(NCORES):
        out[k * BPC:(k + 1) * BPC] = np.asarray(
            res.results[k]["out"]).astype(np.float32)
    return out, res


def kernel(**inputs) -> np.ndarray:
    out, _ = _run(inputs, trace=False)
    return out
